# revision 9
# baseline (speedup 1.0000x reference)
"""Trainium2 Bass kernel for the ACT (Adaptive Computation Time) wrapper.

Data-parallel over batch: 32 examples -> 8 NeuronCores x 4 examples.
Per core: 4096 rows (b,l), H=512, d_ff=2048, 5 ACT steps.

Layout: activations transposed -- [H on partitions (4 chunks of 128), rows on
free dim]. Per core the 4096 rows are processed as 8 tiles of 512 rows; all 5
ACT steps for a tile run back-to-back with state resident in SBUF.

FFN matmuls run in float32r (fp32 with 11 mantissa bits, full PE rate at
N=512); the ponder matvec + halting math stay in full fp32 to keep the
discrete halting decisions faithful to the fp32 reference.

The coordinate embedding (pos_signal + step_signal) and its ponder-logit
contribution are precomputed on the host:
  sR(s)    = ffn_psum(s-1) + [b2 + pos + step(s)]   (one DVE add from PSUM)
  logit(s) = trans(s-1) . w(s) + [(pos + step(s)) . w(s)]
"""
import os
import numpy as np

B, L, H, DFF, S = 32, 1024, 512, 2048, 5
NCORES = 8
BLOC = B // NCORES            # 4 examples per core
ROWS = BLOC * L               # 4096 rows per core
TN = 512                      # rows per tile
NT = ROWS // TN               # 8 tiles
KH = H // 128                 # 4 h-chunks
KF = DFF // 128               # 16 dff-chunks
THRESH = 1.0 - 0.01
MAX_TIMESCALE = 1.0e4

FFN_F32R = os.environ.get("ACT_FFN_DT", "f32r") == "f32r"


def _round_f32r(x, bits=11):
    m, e = np.frexp(x.astype(np.float32))
    m = np.round(m * (1 << (bits + 1))) / (1 << (bits + 1))
    return np.ldexp(m, e).astype(np.float32)


def _timing_signal(positions, channels):
    num_ts = channels // 2
    log_inc = np.log(MAX_TIMESCALE) / max(num_ts - 1, 1)
    inv_ts = np.exp(np.arange(num_ts, dtype=np.float64) * -log_inc)
    scaled = positions[..., None].astype(np.float64) * inv_ts
    return np.concatenate([np.sin(scaled), np.cos(scaled)], axis=-1)


_BUILT = {}


def _build():
    key = ("nc", FFN_F32R)
    if key in _BUILT:
        return _BUILT[key]
    import concourse.bacc as bacc
    import concourse.mybir as mybir
    from concourse.tile import TileContext

    f32 = mybir.dt.float32
    fmm = mybir.dt.float32r if FFN_F32R else f32
    Alu = mybir.AluOpType
    Act = mybir.ActivationFunctionType

    nc = bacc.Bacc("TRN2", target_bir_lowering=False)

    # ---- DRAM parameters (per-core shapes) ----
    xT = nc.declare_dram_parameter("xT", [H, ROWS], f32, isOutput=False)
    # pos+step(+b2) planes: rows s*128..(s+1)*128, cols k*L..(k+1)*L
    psd = nc.declare_dram_parameter("psd", [S * 128, KH * L], f32, isOutput=False)
    lcd = nc.declare_dram_parameter("lcd", [S, L], f32, isOutput=False)
    W1d = nc.declare_dram_parameter("W1r", [H, DFF], fmm, isOutput=False)
    W2d = nc.declare_dram_parameter("W2r", [DFF, H], fmm, isOutput=False)
    b1d = nc.declare_dram_parameter("b1c", [128, KF], f32, isOutput=False)
    b2d = nc.declare_dram_parameter("b2c", [128, KH], f32, isOutput=False)
    pwd = nc.declare_dram_parameter("pwc", [128, S * KH], f32, isOutput=False)
    pbd = nc.declare_dram_parameter("pbb", [1, 8], f32, isOutput=False)
    outT = nc.declare_dram_parameter("outT", [H, ROWS], f32, isOutput=True)
    nu_out = nc.declare_dram_parameter("nu", [1, ROWS], f32, isOutput=True)
    rem_out = nc.declare_dram_parameter("rem", [1, ROWS], f32, isOutput=True)

    with TileContext(nc) as tc:
        with tc.tile_pool(name="const", bufs=1) as cpool, \
             tc.tile_pool(name="work", bufs=2) as wpool, \
             tc.tile_pool(name="hpool", bufs=18) as hpool, \
             tc.tile_pool(name="pp", bufs=1, space="PSUM") as pp, \
             tc.tile_pool(name="ph", bufs=3, space="PSUM") as ph, \
             tc.tile_pool(name="po", bufs=4, space="PSUM") as po:

            # ---- resident constants ----
            W1sb = cpool.tile([128, KH * DFF], fmm, name="W1sb")
            for k in range(KH):
                nc.sync.dma_start(W1sb[:, k * DFF:(k + 1) * DFF],
                                  W1d[k * 128:(k + 1) * 128, :])
            W2sb = cpool.tile([128, KF * H], fmm, name="W2sb")
            for m in range(KF):
                nc.sync.dma_start(W2sb[:, m * H:(m + 1) * H],
                                  W2d[m * 128:(m + 1) * 128, :])
            b1sb = cpool.tile([128, KF], f32, name="b1sb")
            nc.sync.dma_start(b1sb[:], b1d[:])
            b2sb = cpool.tile([128, KH], f32, name="b2sb")
            nc.sync.dma_start(b2sb[:], b2d[:])
            pwsb = cpool.tile([128, S * KH], f32, name="pwsb")
            nc.sync.dma_start(pwsb[:], pwd[:])
            pbsb = cpool.tile([1, 8], f32, name="pbsb")
            nc.sync.dma_start(pbsb[:], pbd[:])

            for r in range(NT):
                par = r % 2  # which half of the L columns this tile covers

                # per-tile halting state [1, TN]
                Ht = wpool.tile([1, TN], f32, name="Ht", tag="Ht")
                Rt = wpool.tile([1, TN], f32, name="Rt", tag="Rt")
                NUt = wpool.tile([1, TN], f32, name="NUt", tag="NUt")
                nc.gpsimd.memset(Ht[:], 0.0)
                nc.gpsimd.memset(Rt[:], 0.0)
                nc.gpsimd.memset(NUt[:], 0.0)

                # state entering step 0 = x tile (fp32: matvec + blend path)
                state = wpool.tile([128, KH * TN], f32, name="state0", tag="state", bufs=3)
                for k in range(KH):
                    nc.sync.dma_start(
                        state[:, k * TN:(k + 1) * TN],
                        xT[k * 128:(k + 1) * 128, r * TN:(r + 1) * TN])

                # pos+step plane for step 0 -> f32r FFN1 input for step 0
                sR = wpool.tile([128, KH * TN], fmm, name="sR0", tag="sR")
                for k in range(KH):
                    ps0 = wpool.tile([128, TN], f32, name="ps0", tag="pstile", bufs=3)
                    nc.sync.dma_start(
                        ps0[:],
                        psd[0:128, k * L + par * TN: k * L + par * TN + TN])
                    nc.vector.tensor_tensor(
                        sR[:, k * TN:(k + 1) * TN],
                        state[:, k * TN:(k + 1) * TN],
                        ps0[:], op=Alu.add)

                new = None
                for s in range(S):
                    # ---- ponder logit matvec (fp32) + const + sigmoid ----
                    lc = wpool.tile([1, TN], f32, name="lc", tag="lc")
                    nc.sync.dma_start(
                        lc[:], lcd[s:s + 1, par * TN: par * TN + TN])
                    pps = pp.tile([1, TN], f32, name="pps", tag="pps")
                    for k in range(KH):
                        nc.tensor.matmul(
                            pps[:],
                            pwsb[:, s * KH + k: s * KH + k + 1],
                            state[:, k * TN:(k + 1) * TN],
                            start=(k == 0), stop=(k == KH - 1))
                    lg = wpool.tile([1, TN], f32, name="lg", tag="lg", bufs=1)
                    nc.vector.tensor_tensor(lg[:], pps[:], lc[:], op=Alu.add)
                    p_sb = wpool.tile([1, TN], f32, name="p_sb", tag="p_sb", bufs=1)
                    nc.scalar.activation(p_sb[:], lg[:], Act.Sigmoid,
                                         bias=pbsb[0:1, s:s + 1])

                    # ---- halting chain (fp32, [1, TN]) ----
                    def vtile(nm):
                        return wpool.tile([1, TN], f32, name=nm, tag=nm, bufs=1)
                    still = vtile("still")
                    nc.vector.tensor_scalar(still[:], Ht[:], 1.0, None, op0=Alu.is_lt)
                    ps_ = vtile("ps_")
                    nc.vector.tensor_tensor(ps_[:], p_sb[:], still[:], op=Alu.mult)
                    hp = vtile("hp")
                    nc.vector.tensor_tensor(hp[:], Ht[:], ps_[:], op=Alu.add)
                    nh = vtile("nh")
                    nc.vector.scalar_tensor_tensor(
                        nh[:], hp[:], THRESH, still[:], op0=Alu.is_gt, op1=Alu.mult)
                    sr = vtile("sr")
                    nc.vector.tensor_tensor(sr[:], still[:], nh[:], op=Alu.subtract)
                    t6 = vtile("t6")
                    nc.vector.tensor_tensor(t6[:], ps_[:], sr[:], op=Alu.mult)
                    nc.vector.tensor_tensor(Ht[:], Ht[:], t6[:], op=Alu.add)
                    t8 = wpool.tile([1, TN], f32, name="t8", tag="ps_", bufs=1)
                    nc.vector.tensor_scalar(t8[:], Ht[:], -1.0, 1.0,
                                            op0=Alu.mult, op1=Alu.add)
                    t9 = wpool.tile([1, TN], f32, name="t9", tag="hp", bufs=1)
                    nc.vector.tensor_tensor(t9[:], t8[:], nh[:], op=Alu.mult)
                    nc.vector.tensor_tensor(Rt[:], Rt[:], t9[:], op=Alu.add)
                    t11 = wpool.tile([1, TN], f32, name="t11", tag="sr", bufs=1)
                    nc.vector.tensor_tensor(t11[:], nh[:], Rt[:], op=Alu.mult)
                    nc.vector.tensor_tensor(Ht[:], Ht[:], t11[:], op=Alu.add)
                    nc.vector.tensor_tensor(NUt[:], NUt[:], still[:], op=Alu.add)
                    uw = vtile("uw")
                    nc.vector.tensor_tensor(uw[:], t6[:], t11[:], op=Alu.add)

                    # ---- broadcast uw to all partitions ----
                    uwb = wpool.tile([128, TN], f32, name="uwb", tag="uwb")
                    nc.gpsimd.partition_broadcast(uwb[:], uw[:])

                    # ---- FFN1 (all m) ----
                    hms = []
                    for m in range(KF):
                        phs = ph.tile([128, TN], f32, name="phs", tag="phs")
                        for k in range(KH):
                            nc.tensor.matmul(
                                phs[:],
                                W1sb[:, k * DFF + m * 128: k * DFF + (m + 1) * 128],
                                sR[:, k * TN:(k + 1) * TN],
                                start=(k == 0), stop=(k == KH - 1))
                        hm = hpool.tile([128, TN], fmm, name="hm", tag="hm")
                        nc.scalar.activation(hm[:], phs[:], Act.Relu,
                                             bias=b1sb[:, m:m + 1])
                        hms.append(hm)

                    # ---- FFN2 j-outer: each j-group closes early ----
                    trans = wpool.tile([128, KH * TN], f32, name="trans", tag="state", bufs=3)
                    if s < S - 1:
                        sRn = wpool.tile([128, KH * TN], fmm, name="sRn", tag="sR")
                    for j in range(KH):
                        poj = po.tile([128, TN], f32, name="poj", tag="po")
                        for m in range(KF):
                            nc.tensor.matmul(
                                poj[:],
                                W2sb[:, m * H + j * 128: m * H + (j + 1) * 128],
                                hms[m][:],
                                start=(m == 0), stop=(m == KF - 1))
                        nc.scalar.activation(
                            trans[:, j * TN:(j + 1) * TN], poj[:],
                            Act.Identity, bias=b2sb[:, j:j + 1])
                        if s < S - 1:
                            pstj = wpool.tile([128, TN], f32, name="pstj", tag="pstile", bufs=3)
                            nc.sync.dma_start(
                                pstj[:],
                                psd[(s + 1) * 128:(s + 2) * 128,
                                    j * L + par * TN: j * L + par * TN + TN])
                            nc.vector.tensor_tensor(
                                sRn[:, j * TN:(j + 1) * TN], poj[:], pstj[:],
                                op=Alu.add)
                    if s < S - 1:
                        sR = sRn

                    # ---- blend: new = prev + (trans - prev) * uw ----
                    if s == 0:
                        new = wpool.tile([128, KH * TN], f32, name="new0", tag="new")
                        for j in range(KH):
                            eng = nc.vector if j % 2 == 0 else nc.gpsimd
                            eng.tensor_tensor(
                                new[:, j * TN:(j + 1) * TN],
                                trans[:, j * TN:(j + 1) * TN], uwb[:],
                                op=Alu.mult)
                    else:
                        for j in range(KH):
                            dj = wpool.tile([128, TN], f32, name="dj", tag="dj")
                            eng = nc.vector if j % 2 == 0 else nc.gpsimd
                            eng.tensor_tensor(
                                dj[:], trans[:, j * TN:(j + 1) * TN],
                                new[:, j * TN:(j + 1) * TN], op=Alu.subtract)
                            eng.tensor_tensor(dj[:], dj[:], uwb[:], op=Alu.mult)
                            eng.tensor_tensor(
                                new[:, j * TN:(j + 1) * TN],
                                new[:, j * TN:(j + 1) * TN], dj[:], op=Alu.add)

                    state = trans  # next step's matvec input

                # ---- tile outputs ----
                for j in range(KH):
                    nc.sync.dma_start(
                        outT[j * 128:(j + 1) * 128, r * TN:(r + 1) * TN],
                        new[:, j * TN:(j + 1) * TN])
                nc.sync.dma_start(nu_out[0:1, r * TN:(r + 1) * TN], NUt[:])
                nc.sync.dma_start(rem_out[0:1, r * TN:(r + 1) * TN], Rt[:])

    nc.finalize()
    _BUILT[key] = nc
    return nc


def _host_inputs(x, ponder_weights, ponder_bias, W1, b1, W2, b2):
    pos_sig = _timing_signal(np.arange(L, dtype=np.float64), H)      # [L, H] f64
    step_sig = _timing_signal(np.arange(S, dtype=np.float64), H)     # [S, H] f64

    psd = np.zeros((S * 128, KH * L), np.float32)
    lcd = np.zeros((S, L), np.float32)
    b2_64 = b2.astype(np.float64)
    for s in range(S):
        plane = pos_sig + step_sig[s][None, :]                       # [L, H] f64
        if s > 0:
            plane = plane + b2_64[None, :]
        planeT = plane.T.astype(np.float32)                          # [H, L]
        for k in range(KH):
            psd[s * 128:(s + 1) * 128, k * L:(k + 1) * L] = \
                planeT[k * 128:(k + 1) * 128, :]
        lcd[s] = ((pos_sig + step_sig[s][None, :])
                  @ ponder_weights[s, :, 0].astype(np.float64)).astype(np.float32)

    pwc = np.zeros((128, S * KH), np.float32)
    for s in range(S):
        for k in range(KH):
            pwc[:, s * KH + k] = ponder_weights[s, k * 128:(k + 1) * 128, 0]
    pbb = np.zeros((1, 8), np.float32)
    pbb[0, :S] = ponder_bias[:, 0]
    b1c = np.ascontiguousarray(b1.reshape(KF, 128).T).astype(np.float32)
    b2c = np.ascontiguousarray(b2.reshape(KH, 128).T).astype(np.float32)
    W1r = _round_f32r(W1) if FFN_F32R else W1.astype(np.float32)
    W2r = _round_f32r(W2) if FFN_F32R else W2.astype(np.float32)
    common = {"psd": psd, "lcd": lcd, "W1r": W1r, "W2r": W2r,
              "b1c": b1c, "b2c": b2c, "pwc": pwc, "pbb": pbb}
    in_maps = []
    for c in range(NCORES):
        xT = np.ascontiguousarray(
            x[c * BLOC:(c + 1) * BLOC].reshape(ROWS, H).T.astype(np.float32))
        in_maps.append({"xT": xT, **common})
    return in_maps


def kernel(x, ponder_weights, ponder_bias, W1, b1, W2, b2):
    from concourse.bass_utils import run_bass_kernel_spmd
    nc = _build()
    in_maps = _host_inputs(np.asarray(x, np.float32),
                           np.asarray(ponder_weights, np.float32),
                           np.asarray(ponder_bias, np.float32),
                           np.asarray(W1, np.float32),
                           np.asarray(b1, np.float32),
                           np.asarray(W2, np.float32),
                           np.asarray(b2, np.float32))
    res = run_bass_kernel_spmd(nc, in_maps, core_ids=list(range(NCORES)))
    outs = res.results
    new_state = np.empty((B, L, H), np.float32)
    n_updates = np.empty((B, L), np.float32)
    remainders = np.empty((B, L), np.float32)
    for c in range(NCORES):
        o = outs[c]
        new_state[c * BLOC:(c + 1) * BLOC] = o["outT"].T.reshape(BLOC, L, H)
        n_updates[c * BLOC:(c + 1) * BLOC] = o["nu"].reshape(BLOC, L)
        remainders[c * BLOC:(c + 1) * BLOC] = o["rem"].reshape(BLOC, L)
    return new_state, n_updates, remainders


# revision 10
# speedup vs baseline: 1.5206x; 1.5206x over previous
"""Trainium2 Bass kernel for the ACT (Adaptive Computation Time) wrapper.

Data-parallel over batch: 32 examples -> 8 NeuronCores x 4 examples.
Per core: 4096 rows (b,l), H=512, d_ff=2048, 5 ACT steps.

Layout: activations transposed -- [H on partitions (4 chunks of 128), rows on
free dim]. Per core the 4096 rows are processed as 8 tiles of 512 rows; all 5
ACT steps for a tile run back-to-back with state resident in SBUF.

FFN matmuls run in float32r (fp32 with 11 mantissa bits, full PE rate at
N=512); the ponder matvec + halting math stay in full fp32 to keep the
discrete halting decisions faithful to the fp32 reference.

The coordinate embedding (pos_signal + step_signal) and its ponder-logit
contribution are precomputed on the host:
  sR(s)    = ffn_psum(s-1) + [b2 + pos + step(s)]   (one DVE add from PSUM)
  logit(s) = trans(s-1) . w(s) + [(pos + step(s)) . w(s)]
"""
import os
import numpy as np

B, L, H, DFF, S = 32, 1024, 512, 2048, 5
NCORES = 8
BLOC = B // NCORES            # 4 examples per core
ROWS = BLOC * L               # 4096 rows per core
TN = 512                      # rows per tile
NT = ROWS // TN               # 8 tiles
KH = H // 128                 # 4 h-chunks
KF = DFF // 128               # 16 dff-chunks
THRESH = 1.0 - 0.01
MAX_TIMESCALE = 1.0e4

FFN_F32R = os.environ.get("ACT_FFN_DT", "f32r") == "f32r"


def _round_f32r(x, bits=11):
    m, e = np.frexp(x.astype(np.float32))
    m = np.round(m * (1 << (bits + 1))) / (1 << (bits + 1))
    return np.ldexp(m, e).astype(np.float32)


def _timing_signal(positions, channels):
    num_ts = channels // 2
    log_inc = np.log(MAX_TIMESCALE) / max(num_ts - 1, 1)
    inv_ts = np.exp(np.arange(num_ts, dtype=np.float64) * -log_inc)
    scaled = positions[..., None].astype(np.float64) * inv_ts
    return np.concatenate([np.sin(scaled), np.cos(scaled)], axis=-1)


_BUILT = {}


def _build():
    key = ("nc", FFN_F32R)
    if key in _BUILT:
        return _BUILT[key]
    import concourse.bacc as bacc
    import concourse.mybir as mybir
    from concourse.tile import TileContext

    f32 = mybir.dt.float32
    fmm = mybir.dt.float32r if FFN_F32R else f32
    Alu = mybir.AluOpType
    Act = mybir.ActivationFunctionType

    nc = bacc.Bacc("TRN2", target_bir_lowering=False)

    # ---- DRAM parameters (per-core shapes) ----
    xT = nc.declare_dram_parameter("xT", [H, ROWS], f32, isOutput=False)
    # pos+step(+b2) planes: rows s*128..(s+1)*128, cols k*L..(k+1)*L
    psd = nc.declare_dram_parameter("psd", [S * 128, KH * L], f32, isOutput=False)
    lcd = nc.declare_dram_parameter("lcd", [S, L], f32, isOutput=False)
    W1d = nc.declare_dram_parameter("W1r", [H, DFF], fmm, isOutput=False)
    W2d = nc.declare_dram_parameter("W2r", [DFF, H], fmm, isOutput=False)
    b1d = nc.declare_dram_parameter("b1c", [128, KF], f32, isOutput=False)
    b2d = nc.declare_dram_parameter("b2c", [128, KH], f32, isOutput=False)
    pwd = nc.declare_dram_parameter("pwc", [128, S * KH], f32, isOutput=False)
    pbd = nc.declare_dram_parameter("pbb", [1, 8], f32, isOutput=False)
    outT = nc.declare_dram_parameter("outT", [H, ROWS], f32, isOutput=True)
    nu_out = nc.declare_dram_parameter("nu", [1, ROWS], f32, isOutput=True)
    rem_out = nc.declare_dram_parameter("rem", [1, ROWS], f32, isOutput=True)

    with TileContext(nc) as tc:
        with tc.tile_pool(name="const", bufs=1) as cpool, \
             tc.tile_pool(name="work", bufs=2) as wpool, \
             tc.tile_pool(name="hpool", bufs=18) as hpool, \
             tc.tile_pool(name="pp", bufs=1, space="PSUM") as pp, \
             tc.tile_pool(name="ph", bufs=3, space="PSUM") as ph, \
             tc.tile_pool(name="po", bufs=4, space="PSUM") as po:

            # ---- resident constants ----
            W1sb = cpool.tile([128, KH * DFF], fmm, name="W1sb")
            for k in range(KH):
                nc.sync.dma_start(W1sb[:, k * DFF:(k + 1) * DFF],
                                  W1d[k * 128:(k + 1) * 128, :])
            W2sb = cpool.tile([128, KF * H], fmm, name="W2sb")
            for m in range(KF):
                nc.sync.dma_start(W2sb[:, m * H:(m + 1) * H],
                                  W2d[m * 128:(m + 1) * 128, :])
            b1sb = cpool.tile([128, KF], f32, name="b1sb")
            nc.sync.dma_start(b1sb[:], b1d[:])
            b2sb = cpool.tile([128, KH], f32, name="b2sb")
            nc.sync.dma_start(b2sb[:], b2d[:])
            pwsb = cpool.tile([128, S * KH], f32, name="pwsb")
            nc.sync.dma_start(pwsb[:], pwd[:])
            pbsb = cpool.tile([1, 8], f32, name="pbsb")
            nc.sync.dma_start(pbsb[:], pbd[:])

            for r in range(NT):
                par = r % 2  # which half of the L columns this tile covers

                # per-tile halting state [1, TN]
                Ht = wpool.tile([1, TN], f32, name="Ht", tag="Ht")
                Rt = wpool.tile([1, TN], f32, name="Rt", tag="Rt")
                NUt = wpool.tile([1, TN], f32, name="NUt", tag="NUt")
                nc.gpsimd.memset(Ht[:], 0.0)
                nc.gpsimd.memset(Rt[:], 0.0)
                nc.gpsimd.memset(NUt[:], 0.0)

                # state entering step 0 = x tile (fp32: matvec + blend path)
                state = wpool.tile([128, KH * TN], f32, name="state0", tag="state", bufs=3)
                for k in range(KH):
                    nc.sync.dma_start(
                        state[:, k * TN:(k + 1) * TN],
                        xT[k * 128:(k + 1) * 128, r * TN:(r + 1) * TN])

                # pos+step plane for step 0 -> f32r FFN1 input for step 0
                sR = wpool.tile([128, KH * TN], fmm, name="sR0", tag="sR")
                for k in range(KH):
                    ps0 = wpool.tile([128, TN], f32, name="ps0", tag="pstile", bufs=3)
                    nc.sync.dma_start(
                        ps0[:],
                        psd[0:128, k * L + par * TN: k * L + par * TN + TN])
                    nc.vector.tensor_tensor(
                        sR[:, k * TN:(k + 1) * TN],
                        state[:, k * TN:(k + 1) * TN],
                        ps0[:], op=Alu.add)

                hv = {"state": state, "sR": sR, "new": None, "cnt": None}

                def emit_step(s):
                    state = hv["state"]
                    sR = hv["sR"]
                    new = hv["new"]
                    # ---- ponder logit matvec (fp32) + const + sigmoid ----
                    lc = wpool.tile([1, TN], f32, name="lc", tag="lc")
                    nc.sync.dma_start(
                        lc[:], lcd[s:s + 1, par * TN: par * TN + TN])
                    pps = pp.tile([1, TN], f32, name="pps", tag="pps")
                    for k in range(KH):
                        nc.tensor.matmul(
                            pps[:],
                            pwsb[:, s * KH + k: s * KH + k + 1],
                            state[:, k * TN:(k + 1) * TN],
                            start=(k == 0), stop=(k == KH - 1))
                    lg = wpool.tile([1, TN], f32, name="lg", tag="lg", bufs=1)
                    nc.vector.tensor_tensor(lg[:], pps[:], lc[:], op=Alu.add)
                    p_sb = wpool.tile([1, TN], f32, name="p_sb", tag="p_sb", bufs=1)
                    nc.scalar.activation(p_sb[:], lg[:], Act.Sigmoid,
                                         bias=pbsb[0:1, s:s + 1])

                    # ---- halting chain (fp32, [1, TN]) ----
                    def vtile(nm):
                        return wpool.tile([1, TN], f32, name=nm, tag=nm, bufs=1)
                    still = vtile("still")
                    nc.vector.tensor_scalar(still[:], Ht[:], 1.0, None, op0=Alu.is_lt)
                    ps_ = vtile("ps_")
                    nc.vector.tensor_tensor(ps_[:], p_sb[:], still[:], op=Alu.mult)
                    hp = vtile("hp")
                    nc.vector.tensor_tensor(hp[:], Ht[:], ps_[:], op=Alu.add)
                    nh = vtile("nh")
                    nc.vector.scalar_tensor_tensor(
                        nh[:], hp[:], THRESH, still[:], op0=Alu.is_gt, op1=Alu.mult)
                    sr = vtile("sr")
                    nc.vector.tensor_tensor(sr[:], still[:], nh[:], op=Alu.subtract)
                    if s == 2:
                        cnt = wpool.tile([1, 8], f32, name="cnt", tag="cnt")
                        nc.vector.tensor_reduce(
                            cnt[0:1, 0:1], sr[:], axis=mybir.AxisListType.X,
                            op=Alu.add)
                        hv["cnt"] = cnt
                    t6 = vtile("t6")
                    nc.vector.tensor_tensor(t6[:], ps_[:], sr[:], op=Alu.mult)
                    nc.vector.tensor_tensor(Ht[:], Ht[:], t6[:], op=Alu.add)
                    t8 = wpool.tile([1, TN], f32, name="t8", tag="ps_", bufs=1)
                    nc.vector.tensor_scalar(t8[:], Ht[:], -1.0, 1.0,
                                            op0=Alu.mult, op1=Alu.add)
                    t9 = wpool.tile([1, TN], f32, name="t9", tag="hp", bufs=1)
                    nc.vector.tensor_tensor(t9[:], t8[:], nh[:], op=Alu.mult)
                    nc.vector.tensor_tensor(Rt[:], Rt[:], t9[:], op=Alu.add)
                    t11 = wpool.tile([1, TN], f32, name="t11", tag="sr", bufs=1)
                    nc.vector.tensor_tensor(t11[:], nh[:], Rt[:], op=Alu.mult)
                    nc.vector.tensor_tensor(Ht[:], Ht[:], t11[:], op=Alu.add)
                    nc.vector.tensor_tensor(NUt[:], NUt[:], still[:], op=Alu.add)
                    uw = vtile("uw")
                    nc.vector.tensor_tensor(uw[:], t6[:], t11[:], op=Alu.add)

                    # ---- broadcast uw to all partitions ----
                    uwb = wpool.tile([128, TN], f32, name="uwb", tag="uwb")
                    nc.gpsimd.partition_broadcast(uwb[:], uw[:])

                    # ---- FFN1 (all m) ----
                    hms = []
                    for m in range(KF):
                        phs = ph.tile([128, TN], f32, name="phs", tag="phs")
                        for k in range(KH):
                            nc.tensor.matmul(
                                phs[:],
                                W1sb[:, k * DFF + m * 128: k * DFF + (m + 1) * 128],
                                sR[:, k * TN:(k + 1) * TN],
                                start=(k == 0), stop=(k == KH - 1))
                        hm = hpool.tile([128, TN], fmm, name="hm", tag="hm")
                        nc.scalar.activation(hm[:], phs[:], Act.Relu,
                                             bias=b1sb[:, m:m + 1])
                        hms.append(hm)

                    # ---- FFN2 j-outer: each j-group closes early ----
                    trans = wpool.tile([128, KH * TN], f32, name="trans", tag="state", bufs=3)
                    if s < S - 1:
                        sRn = wpool.tile([128, KH * TN], fmm, name="sRn", tag="sR")
                    for j in range(KH):
                        poj = po.tile([128, TN], f32, name="poj", tag="po")
                        for m in range(KF):
                            nc.tensor.matmul(
                                poj[:],
                                W2sb[:, m * H + j * 128: m * H + (j + 1) * 128],
                                hms[m][:],
                                start=(m == 0), stop=(m == KF - 1))
                        nc.scalar.activation(
                            trans[:, j * TN:(j + 1) * TN], poj[:],
                            Act.Identity, bias=b2sb[:, j:j + 1])
                        if s < S - 1:
                            pstj = wpool.tile([128, TN], f32, name="pstj", tag="pstile", bufs=3)
                            nc.sync.dma_start(
                                pstj[:],
                                psd[(s + 1) * 128:(s + 2) * 128,
                                    j * L + par * TN: j * L + par * TN + TN])
                            nc.vector.tensor_tensor(
                                sRn[:, j * TN:(j + 1) * TN], poj[:], pstj[:],
                                op=Alu.add)
                    if s < S - 1:
                        hv["sR"] = sRn

                    # ---- blend: new = prev + (trans - prev) * uw ----
                    if s == 0:
                        new = wpool.tile([128, KH * TN], f32, name="new0", tag="new")
                        hv["new"] = new
                        for j in range(KH):
                            eng = nc.vector if j % 2 == 0 else nc.gpsimd
                            eng.tensor_tensor(
                                new[:, j * TN:(j + 1) * TN],
                                trans[:, j * TN:(j + 1) * TN], uwb[:],
                                op=Alu.mult)
                    else:
                        for j in range(KH):
                            dj = wpool.tile([128, TN], f32, name="dj", tag="dj")
                            eng = nc.vector if j % 2 == 0 else nc.gpsimd
                            eng.tensor_tensor(
                                dj[:], trans[:, j * TN:(j + 1) * TN],
                                new[:, j * TN:(j + 1) * TN], op=Alu.subtract)
                            eng.tensor_tensor(dj[:], dj[:], uwb[:], op=Alu.mult)
                            eng.tensor_tensor(
                                new[:, j * TN:(j + 1) * TN],
                                new[:, j * TN:(j + 1) * TN], dj[:], op=Alu.add)

                    hv["state"] = trans  # next step's matvec input


                for s in range(3):
                    emit_step(s)

                # all rows halted -> steps 3-4 are no-ops; skip them
                cntr = nc.alloc_registers(f"cntr_{r}")
                nc.regs_load(cntr, hv["cnt"][0:1, 0:1].bitcast(mybir.dt.int32))
                rv = nc.snap(cntr, donate=True)
                with tc.If(rv > 0):
                    emit_step(3)
                    emit_step(4)
                new = hv["new"]

                # ---- tile outputs ----
                for j in range(KH):
                    nc.sync.dma_start(
                        outT[j * 128:(j + 1) * 128, r * TN:(r + 1) * TN],
                        new[:, j * TN:(j + 1) * TN])
                nc.sync.dma_start(nu_out[0:1, r * TN:(r + 1) * TN], NUt[:])
                nc.sync.dma_start(rem_out[0:1, r * TN:(r + 1) * TN], Rt[:])

    nc.finalize()
    _BUILT[key] = nc
    return nc


def _host_inputs(x, ponder_weights, ponder_bias, W1, b1, W2, b2):
    pos_sig = _timing_signal(np.arange(L, dtype=np.float64), H)      # [L, H] f64
    step_sig = _timing_signal(np.arange(S, dtype=np.float64), H)     # [S, H] f64

    psd = np.zeros((S * 128, KH * L), np.float32)
    lcd = np.zeros((S, L), np.float32)
    b2_64 = b2.astype(np.float64)
    for s in range(S):
        plane = pos_sig + step_sig[s][None, :]                       # [L, H] f64
        if s > 0:
            plane = plane + b2_64[None, :]
        planeT = plane.T.astype(np.float32)                          # [H, L]
        for k in range(KH):
            psd[s * 128:(s + 1) * 128, k * L:(k + 1) * L] = \
                planeT[k * 128:(k + 1) * 128, :]
        lcd[s] = ((pos_sig + step_sig[s][None, :])
                  @ ponder_weights[s, :, 0].astype(np.float64)).astype(np.float32)

    pwc = np.zeros((128, S * KH), np.float32)
    for s in range(S):
        for k in range(KH):
            pwc[:, s * KH + k] = ponder_weights[s, k * 128:(k + 1) * 128, 0]
    pbb = np.zeros((1, 8), np.float32)
    pbb[0, :S] = ponder_bias[:, 0]
    b1c = np.ascontiguousarray(b1.reshape(KF, 128).T).astype(np.float32)
    b2c = np.ascontiguousarray(b2.reshape(KH, 128).T).astype(np.float32)
    W1r = _round_f32r(W1) if FFN_F32R else W1.astype(np.float32)
    W2r = _round_f32r(W2) if FFN_F32R else W2.astype(np.float32)
    common = {"psd": psd, "lcd": lcd, "W1r": W1r, "W2r": W2r,
              "b1c": b1c, "b2c": b2c, "pwc": pwc, "pbb": pbb}
    in_maps = []
    for c in range(NCORES):
        xT = np.ascontiguousarray(
            x[c * BLOC:(c + 1) * BLOC].reshape(ROWS, H).T.astype(np.float32))
        in_maps.append({"xT": xT, **common})
    return in_maps


def kernel(x, ponder_weights, ponder_bias, W1, b1, W2, b2):
    from concourse.bass_utils import run_bass_kernel_spmd
    nc = _build()
    in_maps = _host_inputs(np.asarray(x, np.float32),
                           np.asarray(ponder_weights, np.float32),
                           np.asarray(ponder_bias, np.float32),
                           np.asarray(W1, np.float32),
                           np.asarray(b1, np.float32),
                           np.asarray(W2, np.float32),
                           np.asarray(b2, np.float32))
    res = run_bass_kernel_spmd(nc, in_maps, core_ids=list(range(NCORES)))
    outs = res.results
    new_state = np.empty((B, L, H), np.float32)
    n_updates = np.empty((B, L), np.float32)
    remainders = np.empty((B, L), np.float32)
    for c in range(NCORES):
        o = outs[c]
        new_state[c * BLOC:(c + 1) * BLOC] = o["outT"].T.reshape(BLOC, L, H)
        n_updates[c * BLOC:(c + 1) * BLOC] = o["nu"].reshape(BLOC, L)
        remainders[c * BLOC:(c + 1) * BLOC] = o["rem"].reshape(BLOC, L)
    return new_state, n_updates, remainders
